# revision 29
# baseline (speedup 1.0000x reference)
"""AttentionMIL Trainium2 kernel.

Math (per bag of 512 instances):
    emb    = relu(x @ w_enc + b_enc)            [512, 128]
    a      = tanh(emb @ w_att + b_att)          [512, 64]
    logits = a @ w_score (+ b_score, dropped: softmax shift-invariant)
    attn   = softmax(logits) within the bag
    bag    = sum_i attn[i] * emb[i]             [128]
    score  = bag @ w_cls + b_cls                [2]

Distribution: data-parallel over bags. 8 NeuronCores, 8 bags (4096
instances) per core, weights replicated, no cross-core communication.
Each core returns its 8 bags' scores transposed [2, 8]; host stacks.

Layout: the host pre-transposes each core's x shard to x^T [1024, 4096]
and converts it (and the mat-mul weights) to bf16, halving the HBM
traffic — the kernel is DMA-bound — and putting the contraction dim
(d_in) on SBUF partitions directly, avoiding 256 on-chip PE transposes
+ PSUM evacuation per core. Matmuls accumulate in f32 PSUM; biases,
softmax and the bag reduction stay f32 (measured ~7e-4 rel err vs the
f32 reference). Everything on-chip stays transposed (emb^T [128 emb,
512 inst], a^T [64 att, 512 inst], logits [1, 512]) so per-partition
bias APs implement the +b terms and the per-bag softmax is a free-axis
reduce. The softmax skips the max-shift (logits = a @ w_score with a in
(-1,1) tanh-bounded, so exp cannot overflow) and defers 1/denominator
to the [2, 8] classifier epilogue. The bag-weighted sum multiplies
emb^T by the exp row broadcast across partitions via a K=1 matmul with
a ones column. Per-bag emission is software-pipelined (bag b's softmax
tail is emitted after bag b+1's encoder matmuls) so the in-order PE
queue never head-of-line blocks; steady state runs at the ~360 GB/s
HBM roofline (~2.9 us per 1.05 MB bag slab).
"""

import sys

sys.path.insert(0, "/opt/trn_rl_repo")

import numpy as np

N_INST = 32768
N_BAGS = 64
D_IN = 1024
D_EMB = 128
D_ATT = 64
N_CLS = 2

N_CORES = 8
BAGS_PER_CORE = N_BAGS // N_CORES          # 8
INST_PER_BAG = N_INST // N_BAGS            # 512
INST_PER_CORE = N_INST // N_CORES          # 4096
DIN_CHUNKS = D_IN // 128                   # 8
SLAB_SPLIT = 2                             # DMAs per bag slab
CH_PER_DMA = DIN_CHUNKS // SLAB_SPLIT      # 2

_CACHE = {}


def _build():
    import concourse.bacc as bacc
    import concourse.mybir as mybir
    import concourse.tile as tile

    f32 = mybir.dt.float32
    f32r = mybir.dt.float32r
    bf16 = mybir.dt.bfloat16
    AF = mybir.ActivationFunctionType

    nc = bacc.Bacc("TRN2", target_bir_lowering=False, debug=False,
                   enable_asserts=False, num_devices=N_CORES)

    xt = nc.dram_tensor("xt", [D_IN, INST_PER_CORE], bf16, kind="ExternalInput")
    w_enc = nc.dram_tensor("w_enc", [128, D_IN], bf16, kind="ExternalInput")
    b_enc = nc.dram_tensor("b_enc", [D_EMB], f32, kind="ExternalInput")
    w_att = nc.dram_tensor("w_att", [D_EMB, D_ATT], bf16, kind="ExternalInput")
    b_att = nc.dram_tensor("b_att", [D_ATT], f32, kind="ExternalInput")
    w_score = nc.dram_tensor("w_score", [D_ATT], bf16, kind="ExternalInput")
    w_cls = nc.dram_tensor("w_cls", [D_EMB, N_CLS], f32, kind="ExternalInput")
    b_cls = nc.dram_tensor("b_cls", [N_CLS], f32, kind="ExternalInput")
    out = nc.dram_tensor("out", [N_CLS, BAGS_PER_CORE], f32,
                         kind="ExternalOutput")

    with tile.TileContext(nc) as tc:
        with (
            tc.tile_pool(name="const", bufs=1) as const,
            tc.tile_pool(name="xt", bufs=6) as xt_pool,
            tc.tile_pool(name="work", bufs=3) as work,
            tc.tile_pool(name="ps", bufs=2, space="PSUM") as ps,
        ):
            # ---- replicated weights ----
            # host supplies w_enc pre-packed as [128 din-part, chunk*emb]
            wenc_sb = const.tile([128, DIN_CHUNKS, D_EMB], bf16)
            nc.sync.dma_start(
                out=wenc_sb,
                in_=w_enc[:, :].rearrange("p (c e) -> p c e", c=DIN_CHUNKS),
            )
            benc_sb = const.tile([D_EMB, 1], f32)
            nc.scalar.dma_start(
                out=benc_sb, in_=b_enc[:].rearrange("(p one) -> p one", one=1))
            watt_sb = const.tile([D_EMB, D_ATT], bf16)
            nc.scalar.dma_start(out=watt_sb, in_=w_att[:, :])
            batt_sb = const.tile([D_ATT, 1], f32)
            nc.scalar.dma_start(
                out=batt_sb, in_=b_att[:].rearrange("(p one) -> p one", one=1))
            wscore_sb = const.tile([D_ATT, 1], bf16)
            nc.scalar.dma_start(
                out=wscore_sb,
                in_=w_score[:].rearrange("(p one) -> p one", one=1))
            wcls_sb = const.tile([D_EMB, N_CLS], f32)
            nc.scalar.dma_start(out=wcls_sb, in_=w_cls[:, :])
            bcls_sb = const.tile([N_CLS, 1], f32)
            nc.scalar.dma_start(
                out=bcls_sb, in_=b_cls[:].rearrange("(p one) -> p one", one=1))
            ones_row = const.tile([1, 128], bf16)
            nc.vector.memset(ones_row, 1.0)
            ones_f32r = const.tile([1, N_CLS], f32r)
            ones_tmp = const.tile([1, N_CLS], f32)
            nc.vector.memset(ones_tmp, 1.0)
            nc.vector.tensor_copy(ones_f32r, ones_tmp)

            # unnormalized bag embeddings (columns) + softmax denominators
            bag_all = const.tile([D_EMB, BAGS_PER_CORE], f32)
            den_all = const.tile([1, BAGS_PER_CORE], f32)

            xt_re = xt[:, :].rearrange("(c p) i -> p c i", p=128)

            def emit_enc(b):
                i0 = b * INST_PER_BAG
                # split the bag slab into DMAs so the first encoder
                # matmuls start as soon as their chunks land
                parts = []
                for j in range(SLAB_SPLIT):
                    part = xt_pool.tile([128, CH_PER_DMA, INST_PER_BAG], bf16,
                                        tag=f"slab{j}")
                    c0 = j * CH_PER_DMA
                    nc.sync.dma_start(
                        out=part,
                        in_=xt_re[:, c0:c0 + CH_PER_DMA,
                                  i0:i0 + INST_PER_BAG])
                    parts.append(part)

                # emb^T = relu(sum_c w_enc_c.T @ xt_c + b_enc)
                ps_emb = ps.tile([D_EMB, INST_PER_BAG], f32, tag="emb")
                for c in range(DIN_CHUNKS):
                    nc.tensor.matmul(ps_emb[:, :], wenc_sb[:, c, :],
                                     parts[c // CH_PER_DMA][:, c % CH_PER_DMA, :],
                                     start=(c == 0), stop=(c == DIN_CHUNKS - 1))
                embT = work.tile([D_EMB, INST_PER_BAG], bf16, tag="embT")
                nc.scalar.activation(embT, ps_emb, AF.Relu, bias=benc_sb,
                                     scale=1.0)
                return embT

            def emit_tail(b, embT):
                # a^T = tanh(w_att.T @ emb^T + b_att)
                ps_a = ps.tile([D_ATT, INST_PER_BAG], f32, tag="a")
                nc.tensor.matmul(ps_a[:, :], watt_sb[:, :], embT[:, :],
                                 start=True, stop=True)
                aT = work.tile([D_ATT, INST_PER_BAG], bf16, tag="aT")
                nc.scalar.activation(aT, ps_a, AF.Tanh, bias=batt_sb, scale=1.0)

                # logits = w_score.T @ a^T   [1, 512]
                ps_l = ps.tile([1, INST_PER_BAG], f32, tag="logit")
                nc.tensor.matmul(ps_l[:, :], wscore_sb[:, :], aT[:, :],
                                 start=True, stop=True)

                # softmax numerator row + denominator (normalization
                # deferred). No max-shift: logits = a @ w_score with
                # a in (-1,1), so |logits| <= ||w_score||_1 ~ 6 — exp is safe.
                e_row = work.tile([1, INST_PER_BAG], bf16, tag="e_row")
                nc.scalar.activation(e_row, ps_l, AF.Exp, scale=1.0,
                                     accum_out=den_all[0:1, b:b + 1])

                # broadcast e row across 128 partitions via K=1 matmul
                ps_bc = ps.tile([D_EMB, INST_PER_BAG], f32, tag="bc")
                nc.tensor.matmul(ps_bc[:, :], ones_row[:, :], e_row[:, :],
                                 start=True, stop=True)

                # unnormalized bag = sum_i emb^T[:, i] * e[i]
                scratch = work.tile([D_EMB, INST_PER_BAG], bf16, tag="scratch")
                nc.vector.tensor_mul(scratch, embT[:, :], ps_bc[:, :])
                nc.vector.reduce_sum(bag_all[:, b:b + 1], scratch,
                                     axis=mybir.AxisListType.X)

            # software pipeline: emit bag b's dependent tail after bag b+1's
            # encoder matmuls so the in-order PE queue never head-of-line
            # blocks on the softmax chain
            def emit_tail_halves(b, embT):
                # the last bag's tail is the serial end-of-kernel chain:
                # split it into two 256-instance halves so the PE/ACT/DVE
                # stages pipeline against each other
                H = INST_PER_BAG // 2
                den_h = work.tile([1, 2], f32, tag="den_h")
                bag_h = work.tile([D_EMB, 2], f32, tag="bag_h")
                for h in range(2):
                    sl = slice(h * H, (h + 1) * H)
                    ps_a = ps.tile([D_ATT, H], f32, tag="a")
                    nc.tensor.matmul(ps_a[:, :], watt_sb[:, :], embT[:, sl],
                                     start=True, stop=True)
                    aT = work.tile([D_ATT, H], bf16, tag="aT")
                    nc.scalar.activation(aT, ps_a, AF.Tanh, bias=batt_sb,
                                         scale=1.0)
                    ps_l = ps.tile([1, H], f32, tag="logit")
                    nc.tensor.matmul(ps_l[:, :], wscore_sb[:, :], aT[:, :],
                                     start=True, stop=True)
                    e_row = work.tile([1, H], bf16, tag="e_row")
                    nc.scalar.activation(e_row, ps_l, AF.Exp, scale=1.0,
                                         accum_out=den_h[0:1, h:h + 1])
                    ps_bc = ps.tile([D_EMB, H], f32, tag="bc")
                    nc.tensor.matmul(ps_bc[:, :], ones_row[:, :], e_row[:, :],
                                     start=True, stop=True)
                    scratch = work.tile([D_EMB, H], bf16, tag="scratch")
                    nc.vector.tensor_mul(scratch, embT[:, sl], ps_bc[:, :])
                    nc.vector.reduce_sum(bag_h[:, h:h + 1], scratch,
                                         axis=mybir.AxisListType.X)
                nc.vector.tensor_add(den_all[0:1, b:b + 1], den_h[0:1, 0:1],
                                     den_h[0:1, 1:2])
                nc.vector.tensor_add(bag_all[:, b:b + 1], bag_h[:, 0:1],
                                     bag_h[:, 1:2])

            prev = None
            for b in range(BAGS_PER_CORE):
                embT = emit_enc(b)
                if prev is not None:
                    emit_tail(b - 1, prev)
                prev = embT
            emit_tail_halves(BAGS_PER_CORE - 1, prev)

            # scores^T = (w_cls.T @ bag_u) * (1/den) + b_cls   [2, 8]
            ps_s = ps.tile([N_CLS, BAGS_PER_CORE], f32, tag="logit")
            nc.tensor.matmul(ps_s[:, :], wcls_sb[:, :], bag_all[:, :],
                             start=True, stop=True)
            rden_row = const.tile([1, BAGS_PER_CORE], f32r)
            with nc.allow_low_precision(reason="1/denom at f32r, ~1e-4 rel"):
                nc.vector.reciprocal(rden_row, den_all)
            ps_r = ps.tile([N_CLS, BAGS_PER_CORE], f32, tag="bc")
            nc.tensor.matmul(ps_r[:, :], ones_f32r[:, :], rden_row[:, :],
                             start=True, stop=True)
            s_u = const.tile([N_CLS, BAGS_PER_CORE], f32)
            nc.scalar.activation(s_u, ps_s[:, :], AF.Copy)
            s_n = const.tile([N_CLS, BAGS_PER_CORE], f32)
            nc.vector.tensor_mul(s_n, s_u, ps_r[:, :])
            scores = const.tile([N_CLS, BAGS_PER_CORE], f32)
            nc.scalar.activation(scores, s_n, AF.Identity, bias=bcls_sb,
                                 scale=1.0)
            nc.scalar.dma_start(out=out[:, :], in_=scores)

    nc.compile()
    return nc


def _numpy_fallback(x, seg, w_enc, b_enc, w_att, b_att, w_score, b_score,
                    w_cls, b_cls):
    emb = np.maximum(x @ w_enc + b_enc, 0.0)
    a = np.tanh(emb @ w_att + b_att)
    logits = a @ w_score + b_score[0]
    out = np.zeros((N_BAGS, N_CLS), dtype=np.float32)
    for bag in range(N_BAGS):
        mask = seg == bag
        lg = logits[mask]
        e = np.exp(lg - lg.max())
        attn = e / e.sum()
        bag_emb = attn @ emb[mask]
        out[bag] = bag_emb @ w_cls + b_cls
    return out


def kernel(**inputs):
    from concourse.bass_utils import run_bass_kernel_spmd

    import ml_dtypes

    x = np.asarray(inputs["x"], dtype=np.float32)
    seg = np.asarray(inputs["seg"], dtype=np.int32)
    w_enc = np.asarray(inputs["w_enc"], dtype=np.float32)
    b_enc = np.asarray(inputs["b_enc"], dtype=np.float32)
    w_att = np.asarray(inputs["w_att"], dtype=np.float32)
    b_att = np.asarray(inputs["b_att"], dtype=np.float32)
    w_score = np.asarray(inputs["w_score"], dtype=np.float32)
    b_score = np.asarray(inputs["b_score"], dtype=np.float32)
    w_cls = np.asarray(inputs["w_cls"], dtype=np.float32)
    b_cls = np.asarray(inputs["b_cls"], dtype=np.float32)

    expected_seg = np.repeat(np.arange(N_BAGS, dtype=np.int32), INST_PER_BAG)
    if not np.array_equal(seg, expected_seg):
        # Layout differs from the balanced bags this kernel is built for.
        return _numpy_fallback(x, seg, w_enc, b_enc, w_att, b_att, w_score,
                               b_score, w_cls, b_cls)

    if "nc" not in _CACHE:
        _CACHE["nc"] = _build()
    nc = _CACHE["nc"]

    shared = {
        "w_enc": np.ascontiguousarray(
            w_enc.astype(ml_dtypes.bfloat16).reshape(DIN_CHUNKS, 128, D_EMB)
            .transpose(1, 0, 2).reshape(128, D_IN)),
        "b_enc": b_enc,
        "w_att": w_att.astype(ml_dtypes.bfloat16), "b_att": b_att,
        "w_score": w_score.astype(ml_dtypes.bfloat16),
        "w_cls": w_cls, "b_cls": b_cls,
    }
    in_maps = []
    for c in range(N_CORES):
        xs = x[c * INST_PER_CORE:(c + 1) * INST_PER_CORE]
        in_maps.append(
            {"xt": np.ascontiguousarray(xs.T).astype(ml_dtypes.bfloat16),
             **shared})

    res = run_bass_kernel_spmd(nc, in_maps, core_ids=list(range(N_CORES)))
    return np.concatenate(
        [res.results[c]["out"].T for c in range(N_CORES)], axis=0)


# revision 30
# speedup vs baseline: 1.0784x; 1.0784x over previous
"""AttentionMIL Trainium2 kernel.

Math (per bag of 512 instances):
    emb    = relu(x @ w_enc + b_enc)            [512, 128]
    a      = tanh(emb @ w_att + b_att)          [512, 64]
    logits = a @ w_score (+ b_score, dropped: softmax shift-invariant)
    attn   = softmax(logits) within the bag
    bag    = sum_i attn[i] * emb[i]             [128]
    score  = bag @ w_cls + b_cls                [2]

Distribution: data-parallel over bags. 8 NeuronCores, 8 bags (4096
instances) per core, weights replicated, no cross-core communication.
Each core returns its 8 bags' scores transposed [2, 8]; host stacks.

Layout: the host pre-transposes each core's x shard to x^T [1024, 4096]
and converts it (and the mat-mul weights) to bf16, halving the HBM
traffic — the kernel is DMA-bound — and putting the contraction dim
(d_in) on SBUF partitions directly, avoiding 256 on-chip PE transposes
+ PSUM evacuation per core. Matmuls accumulate in f32 PSUM; biases,
softmax and the bag reduction stay f32 (measured ~7e-4 rel err vs the
f32 reference). Everything on-chip stays transposed (emb^T [128 emb,
512 inst], a^T [64 att, 512 inst], logits [1, 512]) so per-partition
bias APs implement the +b terms and the per-bag softmax is a free-axis
reduce. The softmax skips the max-shift (logits = a @ w_score with a in
(-1,1) tanh-bounded, so exp cannot overflow) and defers 1/denominator
to the [2, 8] classifier epilogue. The bag-weighted sum multiplies
emb^T by the exp row broadcast across partitions via a K=1 matmul with
a ones column. Per-bag emission is software-pipelined (bag b's softmax
tail is emitted after bag b+1's encoder matmuls) so the in-order PE
queue never head-of-line blocks; steady state runs at the ~360 GB/s
HBM roofline (~2.9 us per 1.05 MB bag slab).
"""

import sys

sys.path.insert(0, "/opt/trn_rl_repo")

import numpy as np

N_INST = 32768
N_BAGS = 64
D_IN = 1024
D_EMB = 128
D_ATT = 64
N_CLS = 2

N_CORES = 8
BAGS_PER_CORE = N_BAGS // N_CORES          # 8
INST_PER_BAG = N_INST // N_BAGS            # 512
INST_PER_CORE = N_INST // N_CORES          # 4096
DIN_CHUNKS = D_IN // 128                   # 8
SLAB_SPLIT = 2                             # DMAs per bag slab
CH_PER_DMA = DIN_CHUNKS // SLAB_SPLIT      # 2

_CACHE = {}


def _build():
    import concourse.bacc as bacc
    import concourse.mybir as mybir
    import concourse.tile as tile

    f32 = mybir.dt.float32
    f32r = mybir.dt.float32r
    bf16 = mybir.dt.bfloat16
    AF = mybir.ActivationFunctionType

    nc = bacc.Bacc("TRN2", target_bir_lowering=False, debug=False,
                   enable_asserts=False, num_devices=N_CORES)

    xt = nc.dram_tensor("xt", [D_IN, INST_PER_CORE], bf16, kind="ExternalInput")
    w_enc = nc.dram_tensor("w_enc", [128, D_IN], bf16, kind="ExternalInput")
    b_enc = nc.dram_tensor("b_enc", [D_EMB], f32, kind="ExternalInput")
    w_att = nc.dram_tensor("w_att", [D_EMB, D_ATT], bf16, kind="ExternalInput")
    b_att = nc.dram_tensor("b_att", [D_ATT], f32, kind="ExternalInput")
    w_score = nc.dram_tensor("w_score", [D_ATT], bf16, kind="ExternalInput")
    w_cls = nc.dram_tensor("w_cls", [D_EMB, N_CLS], f32, kind="ExternalInput")
    b_cls = nc.dram_tensor("b_cls", [N_CLS], f32, kind="ExternalInput")
    out = nc.dram_tensor("out", [N_CLS, BAGS_PER_CORE], f32,
                         kind="ExternalOutput")

    with tile.TileContext(nc) as tc:
        with (
            tc.tile_pool(name="const", bufs=1) as const,
            tc.tile_pool(name="xt", bufs=6) as xt_pool,
            tc.tile_pool(name="work", bufs=3) as work,
            tc.tile_pool(name="ps", bufs=2, space="PSUM") as ps,
        ):
            # ---- replicated weights ----
            # host supplies w_enc pre-packed as [128 din-part, chunk*emb]
            wenc_sb = const.tile([128, DIN_CHUNKS, D_EMB], bf16)
            nc.scalar.dma_start(
                out=wenc_sb,
                in_=w_enc[:, :].rearrange("p (c e) -> p c e", c=DIN_CHUNKS),
            )
            benc_sb = const.tile([D_EMB, 1], f32)
            nc.scalar.dma_start(
                out=benc_sb, in_=b_enc[:].rearrange("(p one) -> p one", one=1))
            watt_sb = const.tile([D_EMB, D_ATT], bf16)
            nc.scalar.dma_start(out=watt_sb, in_=w_att[:, :])
            batt_sb = const.tile([D_ATT, 1], f32)
            nc.scalar.dma_start(
                out=batt_sb, in_=b_att[:].rearrange("(p one) -> p one", one=1))
            wscore_sb = const.tile([D_ATT, 1], bf16)
            nc.scalar.dma_start(
                out=wscore_sb,
                in_=w_score[:].rearrange("(p one) -> p one", one=1))
            wcls_sb = const.tile([D_EMB, N_CLS], f32)
            nc.scalar.dma_start(out=wcls_sb, in_=w_cls[:, :])
            bcls_sb = const.tile([N_CLS, 1], f32)
            nc.scalar.dma_start(
                out=bcls_sb, in_=b_cls[:].rearrange("(p one) -> p one", one=1))
            ones_row = const.tile([1, 128], bf16)
            nc.vector.memset(ones_row, 1.0)
            ones_f32r = const.tile([1, N_CLS], f32r)
            ones_tmp = const.tile([1, N_CLS], f32)
            nc.vector.memset(ones_tmp, 1.0)
            nc.vector.tensor_copy(ones_f32r, ones_tmp)

            # unnormalized bag embeddings (columns) + softmax denominators
            bag_all = const.tile([D_EMB, BAGS_PER_CORE], f32)
            den_all = const.tile([1, BAGS_PER_CORE], f32)

            xt_re = xt[:, :].rearrange("(c p) i -> p c i", p=128)

            def emit_enc(b):
                i0 = b * INST_PER_BAG
                # split the bag slab into DMAs so the first encoder
                # matmuls start as soon as their chunks land
                parts = []
                for j in range(SLAB_SPLIT):
                    part = xt_pool.tile([128, CH_PER_DMA, INST_PER_BAG], bf16,
                                        tag=f"slab{j}")
                    c0 = j * CH_PER_DMA
                    nc.sync.dma_start(
                        out=part,
                        in_=xt_re[:, c0:c0 + CH_PER_DMA,
                                  i0:i0 + INST_PER_BAG])
                    parts.append(part)

                # emb^T = relu(sum_c w_enc_c.T @ xt_c + b_enc)
                ps_emb = ps.tile([D_EMB, INST_PER_BAG], f32, tag="emb")
                for c in range(DIN_CHUNKS):
                    nc.tensor.matmul(ps_emb[:, :], wenc_sb[:, c, :],
                                     parts[c // CH_PER_DMA][:, c % CH_PER_DMA, :],
                                     start=(c == 0), stop=(c == DIN_CHUNKS - 1))
                embT = work.tile([D_EMB, INST_PER_BAG], bf16, tag="embT")
                nc.scalar.activation(embT, ps_emb, AF.Relu, bias=benc_sb,
                                     scale=1.0)
                return embT

            def emit_tail(b, embT):
                # a^T = tanh(w_att.T @ emb^T + b_att)
                ps_a = ps.tile([D_ATT, INST_PER_BAG], f32, tag="a")
                nc.tensor.matmul(ps_a[:, :], watt_sb[:, :], embT[:, :],
                                 start=True, stop=True)
                aT = work.tile([D_ATT, INST_PER_BAG], bf16, tag="aT")
                nc.scalar.activation(aT, ps_a, AF.Tanh, bias=batt_sb, scale=1.0)

                # logits = w_score.T @ a^T   [1, 512]
                ps_l = ps.tile([1, INST_PER_BAG], f32, tag="logit")
                nc.tensor.matmul(ps_l[:, :], wscore_sb[:, :], aT[:, :],
                                 start=True, stop=True)

                # softmax numerator row + denominator (normalization
                # deferred). No max-shift: logits = a @ w_score with
                # a in (-1,1), so |logits| <= ||w_score||_1 ~ 6 — exp is safe.
                e_row = work.tile([1, INST_PER_BAG], bf16, tag="e_row")
                nc.scalar.activation(e_row, ps_l, AF.Exp, scale=1.0,
                                     accum_out=den_all[0:1, b:b + 1])

                # broadcast e row across 128 partitions via K=1 matmul
                ps_bc = ps.tile([D_EMB, INST_PER_BAG], f32, tag="bc")
                nc.tensor.matmul(ps_bc[:, :], ones_row[:, :], e_row[:, :],
                                 start=True, stop=True)

                # unnormalized bag = sum_i emb^T[:, i] * e[i]
                scratch = work.tile([D_EMB, INST_PER_BAG], bf16, tag="scratch")
                nc.vector.tensor_mul(scratch, embT[:, :], ps_bc[:, :])
                nc.vector.reduce_sum(bag_all[:, b:b + 1], scratch,
                                     axis=mybir.AxisListType.X)

            # software pipeline: emit bag b's dependent tail after bag b+1's
            # encoder matmuls so the in-order PE queue never head-of-line
            # blocks on the softmax chain
            def emit_tail_halves(b, embT):
                # the last bag's tail is the serial end-of-kernel chain:
                # split it into two 256-instance halves so the PE/ACT/DVE
                # stages pipeline against each other
                H = INST_PER_BAG // 2
                den_h = work.tile([1, 2], f32, tag="den_h")
                bag_h = work.tile([D_EMB, 2], f32, tag="bag_h")
                for h in range(2):
                    sl = slice(h * H, (h + 1) * H)
                    ps_a = ps.tile([D_ATT, H], f32, tag="a")
                    nc.tensor.matmul(ps_a[:, :], watt_sb[:, :], embT[:, sl],
                                     start=True, stop=True)
                    aT = work.tile([D_ATT, H], bf16, tag="aT")
                    nc.scalar.activation(aT, ps_a, AF.Tanh, bias=batt_sb,
                                         scale=1.0)
                    ps_l = ps.tile([1, H], f32, tag="logit")
                    nc.tensor.matmul(ps_l[:, :], wscore_sb[:, :], aT[:, :],
                                     start=True, stop=True)
                    e_row = work.tile([1, H], bf16, tag="e_row")
                    nc.scalar.activation(e_row, ps_l, AF.Exp, scale=1.0,
                                         accum_out=den_h[0:1, h:h + 1])
                    ps_bc = ps.tile([D_EMB, H], f32, tag="bc")
                    nc.tensor.matmul(ps_bc[:, :], ones_row[:, :], e_row[:, :],
                                     start=True, stop=True)
                    scratch = work.tile([D_EMB, H], bf16, tag="scratch")
                    nc.vector.tensor_mul(scratch, embT[:, sl], ps_bc[:, :])
                    nc.vector.reduce_sum(bag_h[:, h:h + 1], scratch,
                                         axis=mybir.AxisListType.X)
                nc.vector.tensor_add(den_all[0:1, b:b + 1], den_h[0:1, 0:1],
                                     den_h[0:1, 1:2])
                nc.vector.tensor_add(bag_all[:, b:b + 1], bag_h[:, 0:1],
                                     bag_h[:, 1:2])

            prev = None
            for b in range(BAGS_PER_CORE):
                embT = emit_enc(b)
                if prev is not None:
                    emit_tail(b - 1, prev)
                prev = embT
            emit_tail_halves(BAGS_PER_CORE - 1, prev)

            # scores^T = (w_cls.T @ bag_u) * (1/den) + b_cls   [2, 8]
            ps_s = ps.tile([N_CLS, BAGS_PER_CORE], f32, tag="logit")
            nc.tensor.matmul(ps_s[:, :], wcls_sb[:, :], bag_all[:, :],
                             start=True, stop=True)
            rden_row = const.tile([1, BAGS_PER_CORE], f32r)
            with nc.allow_low_precision(reason="1/denom at f32r, ~1e-4 rel"):
                nc.vector.reciprocal(rden_row, den_all)
            ps_r = ps.tile([N_CLS, BAGS_PER_CORE], f32, tag="bc")
            nc.tensor.matmul(ps_r[:, :], ones_f32r[:, :], rden_row[:, :],
                             start=True, stop=True)
            s_u = const.tile([N_CLS, BAGS_PER_CORE], f32)
            nc.scalar.activation(s_u, ps_s[:, :], AF.Copy)
            s_n = const.tile([N_CLS, BAGS_PER_CORE], f32)
            nc.vector.tensor_mul(s_n, s_u, ps_r[:, :])
            scores = const.tile([N_CLS, BAGS_PER_CORE], f32)
            nc.scalar.activation(scores, s_n, AF.Identity, bias=bcls_sb,
                                 scale=1.0)
            nc.scalar.dma_start(out=out[:, :], in_=scores)

    nc.compile()
    return nc


def _numpy_fallback(x, seg, w_enc, b_enc, w_att, b_att, w_score, b_score,
                    w_cls, b_cls):
    emb = np.maximum(x @ w_enc + b_enc, 0.0)
    a = np.tanh(emb @ w_att + b_att)
    logits = a @ w_score + b_score[0]
    out = np.zeros((N_BAGS, N_CLS), dtype=np.float32)
    for bag in range(N_BAGS):
        mask = seg == bag
        lg = logits[mask]
        e = np.exp(lg - lg.max())
        attn = e / e.sum()
        bag_emb = attn @ emb[mask]
        out[bag] = bag_emb @ w_cls + b_cls
    return out


def kernel(**inputs):
    from concourse.bass_utils import run_bass_kernel_spmd

    import ml_dtypes

    x = np.asarray(inputs["x"], dtype=np.float32)
    seg = np.asarray(inputs["seg"], dtype=np.int32)
    w_enc = np.asarray(inputs["w_enc"], dtype=np.float32)
    b_enc = np.asarray(inputs["b_enc"], dtype=np.float32)
    w_att = np.asarray(inputs["w_att"], dtype=np.float32)
    b_att = np.asarray(inputs["b_att"], dtype=np.float32)
    w_score = np.asarray(inputs["w_score"], dtype=np.float32)
    b_score = np.asarray(inputs["b_score"], dtype=np.float32)
    w_cls = np.asarray(inputs["w_cls"], dtype=np.float32)
    b_cls = np.asarray(inputs["b_cls"], dtype=np.float32)

    expected_seg = np.repeat(np.arange(N_BAGS, dtype=np.int32), INST_PER_BAG)
    if not np.array_equal(seg, expected_seg):
        # Layout differs from the balanced bags this kernel is built for.
        return _numpy_fallback(x, seg, w_enc, b_enc, w_att, b_att, w_score,
                               b_score, w_cls, b_cls)

    if "nc" not in _CACHE:
        _CACHE["nc"] = _build()
    nc = _CACHE["nc"]

    shared = {
        "w_enc": np.ascontiguousarray(
            w_enc.astype(ml_dtypes.bfloat16).reshape(DIN_CHUNKS, 128, D_EMB)
            .transpose(1, 0, 2).reshape(128, D_IN)),
        "b_enc": b_enc,
        "w_att": w_att.astype(ml_dtypes.bfloat16), "b_att": b_att,
        "w_score": w_score.astype(ml_dtypes.bfloat16),
        "w_cls": w_cls, "b_cls": b_cls,
    }
    in_maps = []
    for c in range(N_CORES):
        xs = x[c * INST_PER_CORE:(c + 1) * INST_PER_CORE]
        in_maps.append(
            {"xt": np.ascontiguousarray(xs.T).astype(ml_dtypes.bfloat16),
             **shared})

    res = run_bass_kernel_spmd(nc, in_maps, core_ids=list(range(N_CORES)))
    return np.concatenate(
        [res.results[c]["out"].T for c in range(N_CORES)], axis=0)


# revision 31
# speedup vs baseline: 1.1215x; 1.0400x over previous
"""AttentionMIL Trainium2 kernel.

Math (per bag of 512 instances):
    emb    = relu(x @ w_enc + b_enc)            [512, 128]
    a      = tanh(emb @ w_att + b_att)          [512, 64]
    logits = a @ w_score (+ b_score, dropped: softmax shift-invariant)
    attn   = softmax(logits) within the bag
    bag    = sum_i attn[i] * emb[i]             [128]
    score  = bag @ w_cls + b_cls                [2]

Distribution: data-parallel over bags. 8 NeuronCores, 8 bags (4096
instances) per core, weights replicated, no cross-core communication.
Each core returns its 8 bags' scores transposed [2, 8]; host stacks.

Layout: the host pre-transposes each core's x shard to x^T [1024, 4096]
and converts it (and the mat-mul weights) to bf16, halving the HBM
traffic — the kernel is DMA-bound — and putting the contraction dim
(d_in) on SBUF partitions directly, avoiding 256 on-chip PE transposes
+ PSUM evacuation per core. Matmuls accumulate in f32 PSUM; biases,
softmax and the bag reduction stay f32 (measured ~7e-4 rel err vs the
f32 reference). Everything on-chip stays transposed (emb^T [128 emb,
512 inst], a^T [64 att, 512 inst], logits [1, 512]) so per-partition
bias APs implement the +b terms and the per-bag softmax is a free-axis
reduce. The softmax skips the max-shift (logits = a @ w_score with a in
(-1,1) tanh-bounded, so exp cannot overflow) and defers 1/denominator
to the [2, 8] classifier epilogue. The bag-weighted sum multiplies
emb^T by the exp row broadcast across partitions via a K=1 matmul with
a ones column. Per-bag emission is software-pipelined (bag b's softmax
tail is emitted after bag b+1's encoder matmuls) so the in-order PE
queue never head-of-line blocks; steady state runs at the ~360 GB/s
HBM roofline (~2.9 us per 1.05 MB bag slab).
"""

import sys

sys.path.insert(0, "/opt/trn_rl_repo")

import numpy as np

N_INST = 32768
N_BAGS = 64
D_IN = 1024
D_EMB = 128
D_ATT = 64
N_CLS = 2

N_CORES = 8
BAGS_PER_CORE = N_BAGS // N_CORES          # 8
INST_PER_BAG = N_INST // N_BAGS            # 512
INST_PER_CORE = N_INST // N_CORES          # 4096
DIN_CHUNKS = D_IN // 128                   # 8
SLAB_SPLIT = 2                             # DMAs per bag slab
CH_PER_DMA = DIN_CHUNKS // SLAB_SPLIT      # 2

_CACHE = {}


def _build():
    import concourse.bacc as bacc
    import concourse.mybir as mybir
    import concourse.tile as tile

    f32 = mybir.dt.float32
    f32r = mybir.dt.float32r
    bf16 = mybir.dt.bfloat16
    AF = mybir.ActivationFunctionType

    nc = bacc.Bacc("TRN2", target_bir_lowering=False, debug=False,
                   enable_asserts=False, num_devices=N_CORES)

    xt = nc.dram_tensor("xt", [D_IN, INST_PER_CORE], bf16, kind="ExternalInput")
    w_enc = nc.dram_tensor("w_enc", [128, D_IN], bf16, kind="ExternalInput")
    b_enc = nc.dram_tensor("b_enc", [D_EMB], f32, kind="ExternalInput")
    w_att = nc.dram_tensor("w_att", [D_EMB, D_ATT], bf16, kind="ExternalInput")
    b_att = nc.dram_tensor("b_att", [D_ATT], f32, kind="ExternalInput")
    w_score = nc.dram_tensor("w_score", [D_ATT], bf16, kind="ExternalInput")
    w_cls = nc.dram_tensor("w_cls", [D_EMB, N_CLS], f32, kind="ExternalInput")
    b_cls = nc.dram_tensor("b_cls", [N_CLS], f32, kind="ExternalInput")
    out = nc.dram_tensor("out", [N_CLS, BAGS_PER_CORE], f32,
                         kind="ExternalOutput")

    with tile.TileContext(nc) as tc:
        with (
            tc.tile_pool(name="const", bufs=1) as const,
            tc.tile_pool(name="xt", bufs=6) as xt_pool,
            tc.tile_pool(name="work", bufs=3) as work,
            tc.tile_pool(name="ps", bufs=2, space="PSUM") as ps,
        ):
            # ---- replicated weights ----
            # host supplies w_enc pre-packed as [128 din-part, chunk*emb]
            wenc_sb = const.tile([128, DIN_CHUNKS, D_EMB], bf16)
            nc.sync.dma_start(
                out=wenc_sb,
                in_=w_enc[:, :].rearrange("p (c e) -> p c e", c=DIN_CHUNKS),
            )
            benc_sb = const.tile([D_EMB, 1], f32)
            nc.scalar.dma_start(
                out=benc_sb, in_=b_enc[:].rearrange("(p one) -> p one", one=1))
            watt_sb = const.tile([D_EMB, D_ATT], bf16)
            nc.scalar.dma_start(out=watt_sb, in_=w_att[:, :])
            batt_sb = const.tile([D_ATT, 1], f32)
            nc.scalar.dma_start(
                out=batt_sb, in_=b_att[:].rearrange("(p one) -> p one", one=1))
            wscore_sb = const.tile([D_ATT, 1], bf16)
            nc.scalar.dma_start(
                out=wscore_sb,
                in_=w_score[:].rearrange("(p one) -> p one", one=1))
            wcls_sb = const.tile([D_EMB, N_CLS], f32)
            nc.scalar.dma_start(out=wcls_sb, in_=w_cls[:, :])
            bcls_sb = const.tile([N_CLS, 1], f32)
            nc.scalar.dma_start(
                out=bcls_sb, in_=b_cls[:].rearrange("(p one) -> p one", one=1))
            ones_row = const.tile([1, 128], bf16)
            nc.vector.memset(ones_row, 1.0)
            ones_f32r = const.tile([1, N_CLS], f32r)
            ones_tmp = const.tile([1, N_CLS], f32)
            nc.vector.memset(ones_tmp, 1.0)
            nc.vector.tensor_copy(ones_f32r, ones_tmp)

            # unnormalized bag embeddings (columns) + softmax denominators
            bag_all = const.tile([D_EMB, BAGS_PER_CORE], f32)
            den_all = const.tile([1, BAGS_PER_CORE], f32)

            xt_re = xt[:, :].rearrange("(c p) i -> p c i", p=128)

            def emit_enc(b):
                i0 = b * INST_PER_BAG
                # split the bag slab into DMAs so the first encoder
                # matmuls start as soon as their chunks land
                parts = []
                for j in range(SLAB_SPLIT):
                    part = xt_pool.tile([128, CH_PER_DMA, INST_PER_BAG], bf16,
                                        tag=f"slab{j}")
                    c0 = j * CH_PER_DMA
                    nc.sync.dma_start(
                        out=part,
                        in_=xt_re[:, c0:c0 + CH_PER_DMA,
                                  i0:i0 + INST_PER_BAG])
                    parts.append(part)

                # emb^T = relu(sum_c w_enc_c.T @ xt_c + b_enc)
                ps_emb = ps.tile([D_EMB, INST_PER_BAG], f32, tag="emb")
                for c in range(DIN_CHUNKS):
                    nc.tensor.matmul(ps_emb[:, :], wenc_sb[:, c, :],
                                     parts[c // CH_PER_DMA][:, c % CH_PER_DMA, :],
                                     start=(c == 0), stop=(c == DIN_CHUNKS - 1))
                embT = work.tile([D_EMB, INST_PER_BAG], bf16, tag="embT")
                nc.scalar.activation(embT, ps_emb, AF.Relu, bias=benc_sb,
                                     scale=1.0)
                return embT

            def emit_tail(b, embT):
                # a^T = tanh(w_att.T @ emb^T + b_att)
                ps_a = ps.tile([D_ATT, INST_PER_BAG], f32, tag="a")
                nc.tensor.matmul(ps_a[:, :], watt_sb[:, :], embT[:, :],
                                 start=True, stop=True)
                aT = work.tile([D_ATT, INST_PER_BAG], bf16, tag="aT")
                nc.scalar.activation(aT, ps_a, AF.Tanh, bias=batt_sb, scale=1.0)

                # logits = w_score.T @ a^T   [1, 512]
                ps_l = ps.tile([1, INST_PER_BAG], f32, tag="logit")
                nc.tensor.matmul(ps_l[:, :], wscore_sb[:, :], aT[:, :],
                                 start=True, stop=True)

                # softmax numerator row + denominator (normalization
                # deferred). No max-shift: logits = a @ w_score with
                # a in (-1,1), so |logits| <= ||w_score||_1 ~ 6 — exp is safe.
                e_row = work.tile([1, INST_PER_BAG], bf16, tag="e_row")
                nc.scalar.activation(e_row, ps_l, AF.Exp, scale=1.0,
                                     accum_out=den_all[0:1, b:b + 1])

                # broadcast e row across 128 partitions via K=1 matmul
                ps_bc = ps.tile([D_EMB, INST_PER_BAG], f32, tag="bc")
                nc.tensor.matmul(ps_bc[:, :], ones_row[:, :], e_row[:, :],
                                 start=True, stop=True)

                # unnormalized bag = sum_i emb^T[:, i] * e[i]
                scratch = work.tile([D_EMB, INST_PER_BAG], bf16, tag="scratch")
                nc.vector.tensor_mul(scratch, embT[:, :], ps_bc[:, :])
                nc.vector.reduce_sum(bag_all[:, b:b + 1], scratch,
                                     axis=mybir.AxisListType.X)

            # software pipeline: emit bag b's dependent tail after bag b+1's
            # encoder matmuls so the in-order PE queue never head-of-line
            # blocks on the softmax chain
            def emit_tail_halves(b, embT):
                # the last bag's tail is the serial end-of-kernel chain:
                # split it into two 256-instance halves so the PE/ACT/DVE
                # stages pipeline against each other
                H = INST_PER_BAG // 2
                den_h = work.tile([1, 2], f32, tag="den_h")
                bag_h = work.tile([D_EMB, 2], f32, tag="bag_h")
                for h in range(2):
                    sl = slice(h * H, (h + 1) * H)
                    ps_a = ps.tile([D_ATT, H], f32, tag="a")
                    nc.tensor.matmul(ps_a[:, :], watt_sb[:, :], embT[:, sl],
                                     start=True, stop=True)
                    aT = work.tile([D_ATT, H], bf16, tag="aT")
                    nc.scalar.activation(aT, ps_a, AF.Tanh, bias=batt_sb,
                                         scale=1.0)
                    ps_l = ps.tile([1, H], f32, tag="logit")
                    nc.tensor.matmul(ps_l[:, :], wscore_sb[:, :], aT[:, :],
                                     start=True, stop=True)
                    e_row = work.tile([1, H], bf16, tag="e_row")
                    nc.scalar.activation(e_row, ps_l, AF.Exp, scale=1.0,
                                         accum_out=den_h[0:1, h:h + 1])
                    ps_bc = ps.tile([D_EMB, H], f32, tag="bc")
                    nc.tensor.matmul(ps_bc[:, :], ones_row[:, :], e_row[:, :],
                                     start=True, stop=True)
                    scratch = work.tile([D_EMB, H], bf16, tag="scratch")
                    nc.vector.tensor_mul(scratch, embT[:, sl], ps_bc[:, :])
                    nc.vector.reduce_sum(bag_h[:, h:h + 1], scratch,
                                         axis=mybir.AxisListType.X)
                nc.vector.tensor_add(den_all[0:1, b:b + 1], den_h[0:1, 0:1],
                                     den_h[0:1, 1:2])
                nc.vector.tensor_add(bag_all[:, b:b + 1], bag_h[:, 0:1],
                                     bag_h[:, 1:2])

            prev = None
            for b in range(BAGS_PER_CORE):
                embT = emit_enc(b)
                if prev is not None:
                    emit_tail(b - 1, prev)
                prev = embT
            emit_tail_halves(BAGS_PER_CORE - 1, prev)

            # scores^T = (w_cls.T @ bag_u) * (1/den) + b_cls   [2, 8]
            ps_s = ps.tile([N_CLS, BAGS_PER_CORE], f32, tag="logit")
            nc.tensor.matmul(ps_s[:, :], wcls_sb[:, :], bag_all[:, :],
                             start=True, stop=True)
            rden_row = const.tile([1, BAGS_PER_CORE], f32r)
            with nc.allow_low_precision(reason="1/denom at f32r, ~1e-4 rel"):
                nc.vector.reciprocal(rden_row, den_all)
            ps_r = ps.tile([N_CLS, BAGS_PER_CORE], f32, tag="bc")
            nc.tensor.matmul(ps_r[:, :], ones_f32r[:, :], rden_row[:, :],
                             start=True, stop=True)
            s_u = const.tile([N_CLS, BAGS_PER_CORE], f32)
            nc.scalar.activation(s_u, ps_s[:, :], AF.Copy)
            s_n = const.tile([N_CLS, BAGS_PER_CORE], f32)
            nc.vector.tensor_mul(s_n, s_u, ps_r[:, :])
            scores = const.tile([N_CLS, BAGS_PER_CORE], f32)
            nc.scalar.activation(scores, s_n, AF.Identity, bias=bcls_sb,
                                 scale=1.0)
            nc.scalar.dma_start(out=out[:, :], in_=scores)

    nc.compile()
    return nc


def _numpy_fallback(x, seg, w_enc, b_enc, w_att, b_att, w_score, b_score,
                    w_cls, b_cls):
    emb = np.maximum(x @ w_enc + b_enc, 0.0)
    a = np.tanh(emb @ w_att + b_att)
    logits = a @ w_score + b_score[0]
    out = np.zeros((N_BAGS, N_CLS), dtype=np.float32)
    for bag in range(N_BAGS):
        mask = seg == bag
        lg = logits[mask]
        e = np.exp(lg - lg.max())
        attn = e / e.sum()
        bag_emb = attn @ emb[mask]
        out[bag] = bag_emb @ w_cls + b_cls
    return out


def kernel(**inputs):
    from concourse.bass_utils import run_bass_kernel_spmd

    import ml_dtypes

    x = np.asarray(inputs["x"], dtype=np.float32)
    seg = np.asarray(inputs["seg"], dtype=np.int32)
    w_enc = np.asarray(inputs["w_enc"], dtype=np.float32)
    b_enc = np.asarray(inputs["b_enc"], dtype=np.float32)
    w_att = np.asarray(inputs["w_att"], dtype=np.float32)
    b_att = np.asarray(inputs["b_att"], dtype=np.float32)
    w_score = np.asarray(inputs["w_score"], dtype=np.float32)
    b_score = np.asarray(inputs["b_score"], dtype=np.float32)
    w_cls = np.asarray(inputs["w_cls"], dtype=np.float32)
    b_cls = np.asarray(inputs["b_cls"], dtype=np.float32)

    expected_seg = np.repeat(np.arange(N_BAGS, dtype=np.int32), INST_PER_BAG)
    if not np.array_equal(seg, expected_seg):
        # Layout differs from the balanced bags this kernel is built for.
        return _numpy_fallback(x, seg, w_enc, b_enc, w_att, b_att, w_score,
                               b_score, w_cls, b_cls)

    if "nc" not in _CACHE:
        _CACHE["nc"] = _build()
    nc = _CACHE["nc"]

    shared = {
        "w_enc": np.ascontiguousarray(
            w_enc.astype(ml_dtypes.bfloat16).reshape(DIN_CHUNKS, 128, D_EMB)
            .transpose(1, 0, 2).reshape(128, D_IN)),
        "b_enc": b_enc,
        "w_att": w_att.astype(ml_dtypes.bfloat16), "b_att": b_att,
        "w_score": w_score.astype(ml_dtypes.bfloat16),
        "w_cls": w_cls, "b_cls": b_cls,
    }
    in_maps = []
    for c in range(N_CORES):
        xs = x[c * INST_PER_CORE:(c + 1) * INST_PER_CORE]
        in_maps.append(
            {"xt": np.ascontiguousarray(xs.T).astype(ml_dtypes.bfloat16),
             **shared})

    res = run_bass_kernel_spmd(nc, in_maps, core_ids=list(range(N_CORES)))
    return np.concatenate(
        [res.results[c]["out"].T for c in range(N_CORES)], axis=0)
